# revision 15
# baseline (speedup 1.0000x reference)
"""Trainium2 Bass kernel for nn_ComplexPtreeLayer (3-level tree message passing).

Math: for the structured inputs produced by the problem's setup_inputs()
(order matrices are cyclic within-group permutations, seg = i//4, B == K == 4),
each tree layer collapses exactly:

    out_g = (sum of the 4 rows in group g) @ (Wzf @ sum_k Wz_k)^T + c
    c     = 4 * (sum_k bz_k @ Wzf^T + bzf)

because summing a group's 4 rows makes every cyclic slot-gather contribute the
same group sum. Chaining 3 levels with Mc = Wzf @ sum_k Wz_k, the per-level
matmuls (left-multiplications) commute with the pools (right-multiplications
by 0/1 block matrices), so the whole network collapses further to a single
matmul against the 64-leaf tree sums:

    out^T = Mc^3 @ pool64(x^T) + c_final * 1^T
    c_final = c + 4*Mc@c + 16*Mc@Mc@c

The kernel verifies the structural assumptions on the actual inputs at run
time and falls back to an exact dense numpy evaluation of the reference
semantics if they do not hold.

Sharding: data-parallel over trees. 65536 leaves / 8 cores = 8192 consecutive
leaves (= 128 whole trees) per core; weights replicated; no collectives.

The kernel is DMA-bound on streaming x, so x is sent to the device as
fp8 e3m4 (1 byte/elem, 512B contiguous rows = full 360 GB/s DMA rate).
Because the device only ever consumes x through the per-tree 64-leaf sums,
the host encodes x with sigma-delta error feedback along each tree's 64
leaves (per hidden column): quantization errors cancel in the pooled sums,
leaving only the final carry (measured end-to-end rel err ~1.4e-3, vs
~1.1e-2 for plain fp8 rounding and 2e-2 tolerance). Mc^3 streams as fp16.

Device layout is "transposed" (hidden dim on partitions): pooling is a PE
matmul with fp8 x tiles as the stationary operand and a [128,2] 0/1 selector
as the moving operand, summing each tree's 64 leaves and transposing in one
pass into PSUM. Per 1024-row supertile (16 trees), the pooled sums are copied
to SBUF as fp16 and immediately pushed through stage 2 (out^T = Mc^3 @ S64T,
bias folded into the PSUM accumulation via a ones-row matmul), so compute and
output DMA pipeline behind the x stream; only the last 16 trees sit in the
tail. The last x load is split into 4 pieces so its pooling overlaps the DMA.
"""

import sys

import numpy as np

for _p in ("/opt/trn_rl_repo",):
    if _p not in sys.path:
        sys.path.append(_p)

H = 512
N0 = 65536
NCORES = 8
ROWS = N0 // NCORES          # 8192 rows per core
G3 = ROWS // 64              # 128 output trees per core
B = 4
K = 4
SROWS = 1024                 # rows per supertile
NS = ROWS // SROWS           # 8 supertiles
TPS = SROWS // 128           # 8 x-tiles per supertile
GP = SROWS // 64             # 16 trees per supertile

_RUNNER = None


def _check_structured(x, Wz, bz, Wzf, bzf, node_idx, order1, order2, order3,
                      seg1, seg2, seg3):
    if node_idx.shape != (N0,) or x.shape != (N0, H):
        return False
    if not np.array_equal(node_idx, np.arange(N0, dtype=node_idx.dtype)):
        return False
    for o, s, n in ((order1, seg1, N0), (order2, seg2, N0 // B),
                    (order3, seg3, N0 // B // B)):
        if o.shape != (K, n) or s.shape != (n,):
            return False
        i = np.arange(n)
        m = np.arange(K)[:, None]
        exp = (i // B) * B + (i[None, :] % B + m) % B + 1
        if not np.array_equal(o, exp.astype(o.dtype)):
            return False
        if not np.array_equal(s, (i // B).astype(s.dtype)):
            return False
    return True


def _fallback(x, Wz, bz, Wzf, bzf, node_idx, order1, order2, order3,
              seg1, seg2, seg3):
    """Exact dense evaluation of the reference semantics (numpy, fp32)."""
    data = x[node_idx]
    for order, seg in ((order1, seg1), (order2, seg2), (order3, seg3)):
        n = order.shape[1]
        padded = np.concatenate([np.zeros((1, H), data.dtype), data], axis=0)
        acc = np.zeros((n, H), np.float32)
        for k in range(K):
            contrib = padded[order[k]] @ Wz[k].T + bz[k]
            contrib[order[k] == 0] = 0.0
            acc += contrib
        z = acc @ Wzf.T + bzf
        out = np.zeros((n // B, H), np.float32)
        np.add.at(out, seg, z)
        data = out
    return data


def _sigma_delta_fp8(x):
    """Encode x [N0, H] as fp8 e3m4 with error feedback along each tree's 64
    leaves (per hidden column), so quantization errors cancel in the 64-leaf
    sums the kernel computes."""
    import ml_dtypes

    e3m4 = ml_dtypes.float8_e3m4
    xr = x.reshape(N0 // 64, 64, H)
    q = np.empty((N0 // 64, 64, H), dtype=e3m4)
    carry = np.zeros((N0 // 64, H), np.float32)
    for i in range(64):
        tmp = xr[:, i, :] + carry
        qi = tmp.astype(e3m4)
        q[:, i, :] = qi
        carry = tmp - qi.astype(np.float32)
    return np.ascontiguousarray(q.reshape(N0, H))


def _build_runner():
    import concourse.bacc as bacc
    import concourse.bass as bass
    import concourse.mybir as mybir
    import concourse.tile as tile

    f32 = mybir.dt.float32
    f16 = mybir.dt.float16
    f8 = mybir.dt.float8e3

    nc = bacc.Bacc("TRN2", target_bir_lowering=False, debug=False,
                   num_devices=NCORES)

    xs = nc.dram_tensor("xs", [ROWS, H], f8, kind="ExternalInput")
    mc3t = nc.dram_tensor("mc3t", [H, H], f16, kind="ExternalInput")
    p4 = nc.dram_tensor("p4", [128, 2], f8, kind="ExternalInput")
    cfo = nc.dram_tensor("cfo", [1, H + GP], f16, kind="ExternalInput")
    out_t = nc.dram_tensor("out_t", [H, G3], f32, kind="ExternalOutput")

    # rows = s*SROWS + t*128 + p  (NS supertiles, one DMA each)
    xs_v = xs.ap().rearrange("(s t p) h -> s p t h", t=TPS, p=128)
    mc3t_v = mc3t.ap().rearrange("(i p) h -> p i h", p=128)
    out_v = out_t.ap().rearrange("(j p) g -> p j g", p=128)

    with tile.TileContext(nc) as tc:
        with (
            tc.tile_pool(name="consts", bufs=1) as consts,
            tc.tile_pool(name="xpool", bufs=NS) as xpool,
            tc.tile_pool(name="acts", bufs=1) as acts,
            tc.tile_pool(name="zout", bufs=NS) as zoutp,
            tc.tile_pool(name="psum1", bufs=4, space=bass.MemorySpace.PSUM) as psum1,
            tc.tile_pool(name="psum2", bufs=4, space=bass.MemorySpace.PSUM) as psum2,
        ):
            # first x load ahead of the consts so the DMA pipe fills at once
            xts = []
            xt0 = xpool.tile([128, TPS, H], f8, tag="xt", name="xt0")
            nc.sync.dma_start(xt0[:], xs_v[0])
            xts.append(xt0)

            # Mc3^T as one 512KB DMA right behind x0 so its DGE pipeline
            # fill hides under x0's transfer and stage-2 g0 is never starved
            mc3t_sb = consts.tile([128, 4, H], f16, tag="mc3t", name="mc3t_sb")
            nc.sync.dma_start(mc3t_sb[:], mc3t_v)
            # 0/1 pooling selector built on-device (saves a DMA slot)
            p4_sb = consts.tile([128, 2], f8, tag="p4", name="p4_sb")
            nc.vector.memset(p4_sb[:], 0.0)
            nc.vector.memset(p4_sb[0:64, 0:1], 1.0)
            nc.vector.memset(p4_sb[64:128, 1:2], 1.0)
            # cf row [1, H] fp16 followed by GP ones (stage-2 bias trick)
            cfo_sb = consts.tile([1, H + GP], f16, tag="cfo", name="cfo_sb")
            nc.sync.dma_start(cfo_sb[:], cfo.ap())

            # S64T[h, tree] = sum of the tree's 64 leaves, transposed, fp16
            # layout [h-in-chunk partition, chunk i, tree]
            s64t = acts.tile([128, 4, G3], f16, tag="s64t", name="s64t")

            LT = TPS - 1
            for s in range(1, NS):
                xt = xpool.tile([128, TPS, H], f8, tag="xt", name=f"xt{s}")
                if s == NS - 1:
                    # split off the final 128 rows: only 2 trees of chain
                    # latency trail the last byte of the x stream
                    nc.sync.dma_start(xt[:, :LT, :], xs_v[s][:, :LT, :])
                    nc.sync.dma_start(xt[:, LT:, :], xs_v[s][:, LT:, :])
                else:
                    nc.sync.dma_start(xt[:], xs_v[s])
                xts.append(xt)

            def pool_tiles(s, t0, t1):
                # per-tree 64-leaf sums + transpose, via PE
                # matmul: out[h, tree2] = sum_p xt[p, h] * p4[p, tree2]
                xt = xts[s]
                ph = psum1.tile([128, 4, GP], f32, tag="s64ps",
                                name=f"s64ps{s}_{t0}")
                for j in range(4):
                    for t in range(t0, t1):
                        nc.tensor.matmul(
                            ph[:, j, 2 * t:2 * t + 2],
                            xt[:, t, j * 128:(j + 1) * 128],
                            p4_sb[:],
                            start=True, stop=True,
                        )
                # PSUM -> SBUF, fp32 -> fp16, one instruction
                nc.vector.tensor_copy(
                    s64t[:, :, s * GP + 2 * t0:s * GP + 2 * t1],
                    ph[:, :, 2 * t0:2 * t1])

            def stage2_group(g0, ng, eng, zname=None):
                # out^T[:, g0:g0+ng] = Mc^3 @ S64T[:, g0:g0+ng] + c_final
                ps = psum2.tile([128, 4, GP], f32, tag="mm", name=f"ps_mm{g0}")
                for j in range(4):
                    # bias: c_final[j*128:(j+1)*128] x ones[ng]
                    nc.tensor.matmul(
                        ps[:, j, :ng],
                        cfo_sb[:, j * 128:(j + 1) * 128],
                        cfo_sb[:, H:H + ng],
                        start=True, stop=False,
                    )
                    for i in range(4):
                        nc.tensor.matmul(
                            ps[:, j, :ng],
                            mc3t_sb[:, i, j * 128:(j + 1) * 128],
                            s64t[:, i, g0:g0 + ng],
                            start=False, stop=(i == 3),
                        )
                # z copy on DVE (lowest PSUM/SBUF access latency); with the
                # zero-skew order the s64/z interleave on DVE never blocks
                z = zoutp.tile([128, 4, GP], f32, tag="z", name=f"z{g0}")
                nc.vector.tensor_copy(z[:, :, :ng], ps[:, :, :ng])
                eng.dma_start(out_v[:, :, g0:g0 + ng], z[:, :, :ng])

            # zero-skew pipeline: stage-2 for a group issues right after its
            # pooling. Out DMAs that fire while x still streams go through the
            # Pool-engine SWDGE path (no HWDGE contention); the late ones
            # (after the x stream ends) ride the then-idle SP HWDGE queue,
            # which is ~0.5us cheaper per issue than SWDGE.
            for s in range(NS - 1):
                pool_tiles(s, 0, TPS)
                stage2_group(s * GP, GP, nc.sync if s >= 5 else nc.gpsimd)
            s = NS - 1
            pool_tiles(s, 0, LT)
            stage2_group(s * GP, 2 * LT, nc.scalar)
            pool_tiles(s, LT, TPS)
            stage2_group(s * GP + 2 * LT, GP - 2 * LT, nc.sync)

    nc.compile()
    return nc


def kernel(x, Wz, bz, Wzf, bzf, node_idx, order1, order2, order3,
           seg1, seg2, seg3):
    x = np.ascontiguousarray(np.asarray(x, dtype=np.float32))
    Wz = np.asarray(Wz, dtype=np.float32)
    bz = np.asarray(bz, dtype=np.float32)
    Wzf = np.asarray(Wzf, dtype=np.float32)
    bzf = np.asarray(bzf, dtype=np.float32)
    node_idx = np.asarray(node_idx)
    orders = [np.asarray(o) for o in (order1, order2, order3)]
    segs = [np.asarray(s) for s in (seg1, seg2, seg3)]

    if not _check_structured(x, Wz, bz, Wzf, bzf, node_idx, *orders, *segs):
        return _fallback(x, Wz, bz, Wzf, bzf, node_idx, *orders, *segs)

    # host-side weight folding (tiny)
    Wsum = Wz.sum(axis=0, dtype=np.float64)
    Mc = Wzf.astype(np.float64) @ Wsum
    c = 4.0 * (bz.sum(axis=0, dtype=np.float64) @ Wzf.astype(np.float64).T
               + bzf.astype(np.float64))
    cfinal = (c + 4.0 * (Mc @ (c + 4.0 * (Mc @ c)))).astype(np.float32)

    Mc3 = (Mc @ Mc @ Mc).astype(np.float32)
    mc3t = np.ascontiguousarray(Mc3.T.astype(np.float16))      # [h_in, h_out]
    xq = _sigma_delta_fp8(x)
    import ml_dtypes
    p4 = np.zeros((128, 2), ml_dtypes.float8_e3m4)
    p4[np.arange(128), np.arange(128) // 64] = 1.0
    cfo = np.concatenate([cfinal, np.ones(GP, np.float32)]).astype(np.float16)
    cfo = np.ascontiguousarray(cfo.reshape(1, H + GP))

    global _RUNNER
    if _RUNNER is None:
        _RUNNER = _build_runner()
    nc = _RUNNER

    try:
        out_g = _run_fast(nc, xq, mc3t, p4, cfo)               # [8, H, G3]
    except Exception:
        from concourse.bass_utils import run_bass_kernel_spmd

        in_maps = [
            {"xs": xq[i * ROWS:(i + 1) * ROWS], "mc3t": mc3t, "p4": p4,
             "cfo": cfo}
            for i in range(NCORES)
        ]
        res = run_bass_kernel_spmd(nc, in_maps, core_ids=list(range(NCORES)))
        out_g = np.stack([r["out_t"] for r in res.results], axis=0)
    out = np.concatenate(
        [np.ascontiguousarray(out_g[i].T) for i in range(NCORES)], axis=0
    )
    return out


_SHARDED = None


def _run_fast(nc, xq, mc3t, p4, cfo):
    """Execute via a cached shard_map'd PJRT callable (one trace/compile,
    reused across calls). Mirrors bass2jax.run_bass_via_pjrt's SPMD path."""
    global _SHARDED
    import jax
    from jax.sharding import Mesh, PartitionSpec
    from concourse import mybir
    from concourse.bass2jax import (_bass_exec_p, install_neuronx_cc_hook,
                                    partition_id_tensor)

    if _SHARDED is None:
        install_neuronx_cc_hook()
        pname = nc.partition_id_tensor.name if nc.partition_id_tensor else None
        in_names, out_names, out_avals = [], [], []
        for alloc in nc.m.functions[0].allocations:
            if not isinstance(alloc, mybir.MemoryLocationSet):
                continue
            name = alloc.memorylocations[0].name
            if alloc.kind == "ExternalInput":
                if name != pname:
                    in_names.append(name)
            elif alloc.kind == "ExternalOutput":
                out_names.append(name)
                out_avals.append(jax.core.ShapedArray(
                    tuple(alloc.tensor_shape), mybir.dt.np(alloc.dtype)))
        n_params = len(in_names)
        in_names_all = list(in_names) + list(out_names)
        if pname is not None:
            in_names_all.append(pname)

        def _body(*args):
            operands = list(args)
            if pname is not None:
                operands.append(partition_id_tensor())
            return tuple(_bass_exec_p.bind(
                *operands,
                out_avals=tuple(out_avals),
                in_names=tuple(in_names_all),
                out_names=tuple(out_names),
                lowering_input_output_aliases=(),
                sim_require_finite=True,
                sim_require_nnan=True,
                nc=nc,
            ))

        devices = jax.devices()[:NCORES]
        mesh = Mesh(np.asarray(devices), ("core",))
        specs = (PartitionSpec("core"),)
        sharded = jax.jit(
            jax.shard_map(_body, mesh=mesh,
                          in_specs=specs * (n_params + len(out_names)),
                          out_specs=specs * len(out_names),
                          check_rep=False),
            keep_unused=True,
        )
        _SHARDED = (sharded, in_names, out_avals)

    sharded, in_names, out_avals = _SHARDED
    per_core = {
        "xs": xq,                                      # concat of shards == xq
        "mc3t": np.concatenate([mc3t] * NCORES, axis=0),
        "p4": np.concatenate([p4] * NCORES, axis=0),
        "cfo": np.concatenate([cfo] * NCORES, axis=0),
    }
    ins = [per_core[n] for n in in_names]
    zeros = [np.zeros((NCORES * a.shape[0], *a.shape[1:]), a.dtype)
             for a in out_avals]
    out_arrs = sharded(*ins, *zeros)
    return np.asarray(out_arrs[0]).reshape(NCORES, H, G3)


# revision 17
# speedup vs baseline: 1.0190x; 1.0190x over previous
"""Trainium2 Bass kernel for nn_ComplexPtreeLayer (3-level tree message passing).

Math: for the structured inputs produced by the problem's setup_inputs()
(order matrices are cyclic within-group permutations, seg = i//4, B == K == 4),
each tree layer collapses exactly:

    out_g = (sum of the 4 rows in group g) @ (Wzf @ sum_k Wz_k)^T + c
    c     = 4 * (sum_k bz_k @ Wzf^T + bzf)

because summing a group's 4 rows makes every cyclic slot-gather contribute the
same group sum. Chaining 3 levels with Mc = Wzf @ sum_k Wz_k, the per-level
matmuls (left-multiplications) commute with the pools (right-multiplications
by 0/1 block matrices), so the whole network collapses further to a single
matmul against the 64-leaf tree sums:

    out^T = Mc^3 @ pool64(x^T) + c_final * 1^T
    c_final = c + 4*Mc@c + 16*Mc@Mc@c

The kernel verifies the structural assumptions on the actual inputs at run
time and falls back to an exact dense numpy evaluation of the reference
semantics if they do not hold.

Sharding: data-parallel over trees. 65536 leaves / 8 cores = 8192 consecutive
leaves (= 128 whole trees) per core; weights replicated; no collectives.

The kernel is DMA-bound on streaming x, so x is sent to the device as
fp8 e3m4 (1 byte/elem, 512B contiguous rows = full 360 GB/s DMA rate).
Because the device only ever consumes x through the per-tree 64-leaf sums,
the host encodes x with sigma-delta error feedback along each tree's 64
leaves (per hidden column): quantization errors cancel in the pooled sums,
leaving only the final carry (measured end-to-end rel err ~1.4e-3, vs
~1.1e-2 for plain fp8 rounding and 2e-2 tolerance). Mc^3 streams as fp16.

Device layout is "transposed" (hidden dim on partitions): pooling is a PE
matmul with fp8 x tiles as the stationary operand and a [128,2] 0/1 selector
as the moving operand, summing each tree's 64 leaves and transposing in one
pass into PSUM. Per 1024-row supertile (16 trees), the pooled sums are copied
to SBUF as fp16 and immediately pushed through stage 2 (out^T = Mc^3 @ S64T,
bias folded into the PSUM accumulation via a ones-row matmul), so compute and
output DMA pipeline behind the x stream; only the last 16 trees sit in the
tail. The last x load is split into 4 pieces so its pooling overlaps the DMA.
"""

import sys

import numpy as np

for _p in ("/opt/trn_rl_repo",):
    if _p not in sys.path:
        sys.path.append(_p)

H = 512
N0 = 65536
NCORES = 8
ROWS = N0 // NCORES          # 8192 rows per core
G3 = ROWS // 64              # 128 output trees per core
B = 4
K = 4
SROWS = 1024                 # rows per supertile
NS = ROWS // SROWS           # 8 supertiles
TPS = SROWS // 128           # 8 x-tiles per supertile
GP = SROWS // 64             # 16 trees per supertile

_RUNNER = None


def _check_structured(x, Wz, bz, Wzf, bzf, node_idx, order1, order2, order3,
                      seg1, seg2, seg3):
    if node_idx.shape != (N0,) or x.shape != (N0, H):
        return False
    if not np.array_equal(node_idx, np.arange(N0, dtype=node_idx.dtype)):
        return False
    for o, s, n in ((order1, seg1, N0), (order2, seg2, N0 // B),
                    (order3, seg3, N0 // B // B)):
        if o.shape != (K, n) or s.shape != (n,):
            return False
        i = np.arange(n)
        m = np.arange(K)[:, None]
        exp = (i // B) * B + (i[None, :] % B + m) % B + 1
        if not np.array_equal(o, exp.astype(o.dtype)):
            return False
        if not np.array_equal(s, (i // B).astype(s.dtype)):
            return False
    return True


def _fallback(x, Wz, bz, Wzf, bzf, node_idx, order1, order2, order3,
              seg1, seg2, seg3):
    """Exact dense evaluation of the reference semantics (numpy, fp32)."""
    data = x[node_idx]
    for order, seg in ((order1, seg1), (order2, seg2), (order3, seg3)):
        n = order.shape[1]
        padded = np.concatenate([np.zeros((1, H), data.dtype), data], axis=0)
        acc = np.zeros((n, H), np.float32)
        for k in range(K):
            contrib = padded[order[k]] @ Wz[k].T + bz[k]
            contrib[order[k] == 0] = 0.0
            acc += contrib
        z = acc @ Wzf.T + bzf
        out = np.zeros((n // B, H), np.float32)
        np.add.at(out, seg, z)
        data = out
    return data


def _sigma_delta_fp8(x):
    """Encode x [N0, H] as fp8 e3m4 with error feedback along each tree's 64
    leaves (per hidden column), so quantization errors cancel in the 64-leaf
    sums the kernel computes."""
    import ml_dtypes

    e3m4 = ml_dtypes.float8_e3m4
    xr = x.reshape(N0 // 64, 64, H)
    q = np.empty((N0 // 64, 64, H), dtype=e3m4)
    carry = np.zeros((N0 // 64, H), np.float32)
    for i in range(64):
        tmp = xr[:, i, :] + carry
        qi = tmp.astype(e3m4)
        q[:, i, :] = qi
        carry = tmp - qi.astype(np.float32)
    return np.ascontiguousarray(q.reshape(N0, H))


def _build_runner():
    import concourse.bacc as bacc
    import concourse.bass as bass
    import concourse.mybir as mybir
    import concourse.tile as tile

    f32 = mybir.dt.float32
    f16 = mybir.dt.float16
    f8 = mybir.dt.float8e3

    nc = bacc.Bacc("TRN2", target_bir_lowering=False, debug=False,
                   num_devices=NCORES)

    xs = nc.dram_tensor("xs", [ROWS, H], f8, kind="ExternalInput")
    mc3t = nc.dram_tensor("mc3t", [H, H], f16, kind="ExternalInput")
    p4 = nc.dram_tensor("p4", [128, 2], f8, kind="ExternalInput")
    cfo = nc.dram_tensor("cfo", [1, H + GP], f16, kind="ExternalInput")
    out_t = nc.dram_tensor("out_t", [H, G3], f32, kind="ExternalOutput")

    # rows = s*SROWS + t*128 + p  (NS supertiles, one DMA each)
    xs_v = xs.ap().rearrange("(s t p) h -> s p t h", t=TPS, p=128)
    mc3t_v = mc3t.ap().rearrange("(i p) h -> p i h", p=128)
    out_v = out_t.ap().rearrange("(j p) g -> p j g", p=128)

    with tile.TileContext(nc) as tc:
        with (
            tc.tile_pool(name="consts", bufs=1) as consts,
            tc.tile_pool(name="xpool", bufs=NS) as xpool,
            tc.tile_pool(name="acts", bufs=1) as acts,
            tc.tile_pool(name="zout", bufs=NS) as zoutp,
            tc.tile_pool(name="psum1", bufs=4, space=bass.MemorySpace.PSUM) as psum1,
            tc.tile_pool(name="psum2", bufs=4, space=bass.MemorySpace.PSUM) as psum2,
        ):
            # first x load ahead of the consts so the DMA pipe fills at once
            xts = []
            xt0 = xpool.tile([128, TPS, H], f8, tag="xt", name="xt0")
            nc.sync.dma_start(xt0[:], xs_v[0])
            xts.append(xt0)

            # Mc3^T as one 512KB DMA right behind x0 so its DGE pipeline
            # fill hides under x0's transfer and stage-2 g0 is never starved
            mc3t_sb = consts.tile([128, 4, H], f16, tag="mc3t", name="mc3t_sb")
            nc.sync.dma_start(mc3t_sb[:], mc3t_v)
            # 0/1 pooling selector built on-device (saves a DMA slot)
            p4_sb = consts.tile([128, 2], f8, tag="p4", name="p4_sb")
            nc.vector.memset(p4_sb[:], 0.0)
            nc.vector.memset(p4_sb[0:64, 0:1], 1.0)
            nc.vector.memset(p4_sb[64:128, 1:2], 1.0)
            # cf row [1, H] fp16 followed by GP ones (stage-2 bias trick)
            cfo_sb = consts.tile([1, H + GP], f16, tag="cfo", name="cfo_sb")
            nc.sync.dma_start(cfo_sb[:], cfo.ap())

            # S64T[h, tree] = sum of the tree's 64 leaves, transposed, fp16
            # layout [h-in-chunk partition, chunk i, tree]
            s64t = acts.tile([128, 4, G3], f16, tag="s64t", name="s64t")

            for s in range(1, NS):
                xt = xpool.tile([128, TPS, H], f8, tag="xt", name=f"xt{s}")
                nc.sync.dma_start(xt[:], xs_v[s])
                xts.append(xt)

            def pool_tiles(s, t0, t1):
                # per-tree 64-leaf sums + transpose, via PE
                # matmul: out[h, tree2] = sum_p xt[p, h] * p4[p, tree2]
                xt = xts[s]
                ph = psum1.tile([128, 4, GP], f32, tag="s64ps",
                                name=f"s64ps{s}_{t0}")
                for j in range(4):
                    for t in range(t0, t1):
                        nc.tensor.matmul(
                            ph[:, j, 2 * t:2 * t + 2],
                            xt[:, t, j * 128:(j + 1) * 128],
                            p4_sb[:],
                            start=True, stop=True,
                        )
                # PSUM -> SBUF, fp32 -> fp16, one instruction
                nc.vector.tensor_copy(
                    s64t[:, :, s * GP + 2 * t0:s * GP + 2 * t1],
                    ph[:, :, 2 * t0:2 * t1])

            def stage2_group(g0, ng, eng, zname=None):
                # out^T[:, g0:g0+ng] = Mc^3 @ S64T[:, g0:g0+ng] + c_final
                ps = psum2.tile([128, 4, GP], f32, tag="mm", name=f"ps_mm{g0}")
                for j in range(4):
                    # bias: c_final[j*128:(j+1)*128] x ones[ng]
                    nc.tensor.matmul(
                        ps[:, j, :ng],
                        cfo_sb[:, j * 128:(j + 1) * 128],
                        cfo_sb[:, H:H + ng],
                        start=True, stop=False,
                    )
                    for i in range(4):
                        nc.tensor.matmul(
                            ps[:, j, :ng],
                            mc3t_sb[:, i, j * 128:(j + 1) * 128],
                            s64t[:, i, g0:g0 + ng],
                            start=False, stop=(i == 3),
                        )
                # z copy on DVE (lowest PSUM/SBUF access latency); with the
                # zero-skew order the s64/z interleave on DVE never blocks
                z = zoutp.tile([128, 4, GP], f32, tag="z", name=f"z{g0}")
                nc.vector.tensor_copy(z[:, :, :ng], ps[:, :, :ng])
                eng.dma_start(out_v[:, :, g0:g0 + ng], z[:, :, :ng])

            # zero-skew pipeline: stage-2 for a group issues right after its
            # pooling. Out DMAs that fire while x still streams go through the
            # Pool-engine SWDGE path (no HWDGE contention); the late ones
            # (after the x stream ends) ride the then-idle SP HWDGE queue,
            # which is ~0.5us cheaper per issue than SWDGE.
            for s in range(NS):
                pool_tiles(s, 0, TPS)
                stage2_group(s * GP, GP, nc.sync if s >= 5 else nc.gpsimd)

    nc.compile()
    return nc


def kernel(x, Wz, bz, Wzf, bzf, node_idx, order1, order2, order3,
           seg1, seg2, seg3):
    x = np.ascontiguousarray(np.asarray(x, dtype=np.float32))
    Wz = np.asarray(Wz, dtype=np.float32)
    bz = np.asarray(bz, dtype=np.float32)
    Wzf = np.asarray(Wzf, dtype=np.float32)
    bzf = np.asarray(bzf, dtype=np.float32)
    node_idx = np.asarray(node_idx)
    orders = [np.asarray(o) for o in (order1, order2, order3)]
    segs = [np.asarray(s) for s in (seg1, seg2, seg3)]

    if not _check_structured(x, Wz, bz, Wzf, bzf, node_idx, *orders, *segs):
        return _fallback(x, Wz, bz, Wzf, bzf, node_idx, *orders, *segs)

    # host-side weight folding (tiny)
    Wsum = Wz.sum(axis=0, dtype=np.float64)
    Mc = Wzf.astype(np.float64) @ Wsum
    c = 4.0 * (bz.sum(axis=0, dtype=np.float64) @ Wzf.astype(np.float64).T
               + bzf.astype(np.float64))
    cfinal = (c + 4.0 * (Mc @ (c + 4.0 * (Mc @ c)))).astype(np.float32)

    Mc3 = (Mc @ Mc @ Mc).astype(np.float32)
    mc3t = np.ascontiguousarray(Mc3.T.astype(np.float16))      # [h_in, h_out]
    xq = _sigma_delta_fp8(x)
    import ml_dtypes
    p4 = np.zeros((128, 2), ml_dtypes.float8_e3m4)
    p4[np.arange(128), np.arange(128) // 64] = 1.0
    cfo = np.concatenate([cfinal, np.ones(GP, np.float32)]).astype(np.float16)
    cfo = np.ascontiguousarray(cfo.reshape(1, H + GP))

    global _RUNNER
    if _RUNNER is None:
        _RUNNER = _build_runner()
    nc = _RUNNER

    try:
        out_g = _run_fast(nc, xq, mc3t, p4, cfo)               # [8, H, G3]
    except Exception:
        from concourse.bass_utils import run_bass_kernel_spmd

        in_maps = [
            {"xs": xq[i * ROWS:(i + 1) * ROWS], "mc3t": mc3t, "p4": p4,
             "cfo": cfo}
            for i in range(NCORES)
        ]
        res = run_bass_kernel_spmd(nc, in_maps, core_ids=list(range(NCORES)))
        out_g = np.stack([r["out_t"] for r in res.results], axis=0)
    out = np.concatenate(
        [np.ascontiguousarray(out_g[i].T) for i in range(NCORES)], axis=0
    )
    return out


_SHARDED = None


def _run_fast(nc, xq, mc3t, p4, cfo):
    """Execute via a cached shard_map'd PJRT callable (one trace/compile,
    reused across calls). Mirrors bass2jax.run_bass_via_pjrt's SPMD path."""
    global _SHARDED
    import jax
    from jax.sharding import Mesh, PartitionSpec
    from concourse import mybir
    from concourse.bass2jax import (_bass_exec_p, install_neuronx_cc_hook,
                                    partition_id_tensor)

    if _SHARDED is None:
        install_neuronx_cc_hook()
        pname = nc.partition_id_tensor.name if nc.partition_id_tensor else None
        in_names, out_names, out_avals = [], [], []
        for alloc in nc.m.functions[0].allocations:
            if not isinstance(alloc, mybir.MemoryLocationSet):
                continue
            name = alloc.memorylocations[0].name
            if alloc.kind == "ExternalInput":
                if name != pname:
                    in_names.append(name)
            elif alloc.kind == "ExternalOutput":
                out_names.append(name)
                out_avals.append(jax.core.ShapedArray(
                    tuple(alloc.tensor_shape), mybir.dt.np(alloc.dtype)))
        n_params = len(in_names)
        in_names_all = list(in_names) + list(out_names)
        if pname is not None:
            in_names_all.append(pname)

        def _body(*args):
            operands = list(args)
            if pname is not None:
                operands.append(partition_id_tensor())
            return tuple(_bass_exec_p.bind(
                *operands,
                out_avals=tuple(out_avals),
                in_names=tuple(in_names_all),
                out_names=tuple(out_names),
                lowering_input_output_aliases=(),
                sim_require_finite=True,
                sim_require_nnan=True,
                nc=nc,
            ))

        devices = jax.devices()[:NCORES]
        mesh = Mesh(np.asarray(devices), ("core",))
        specs = (PartitionSpec("core"),)
        sharded = jax.jit(
            jax.shard_map(_body, mesh=mesh,
                          in_specs=specs * (n_params + len(out_names)),
                          out_specs=specs * len(out_names),
                          check_rep=False),
            keep_unused=True,
        )
        _SHARDED = (sharded, in_names, out_avals)

    sharded, in_names, out_avals = _SHARDED
    per_core = {
        "xs": xq,                                      # concat of shards == xq
        "mc3t": np.concatenate([mc3t] * NCORES, axis=0),
        "p4": np.concatenate([p4] * NCORES, axis=0),
        "cfo": np.concatenate([cfo] * NCORES, axis=0),
    }
    ins = [per_core[n] for n in in_names]
    zeros = [np.zeros((NCORES * a.shape[0], *a.shape[1:]), a.dtype)
             for a in out_avals]
    out_arrs = sharded(*ins, *zeros)
    return np.asarray(out_arrs[0]).reshape(NCORES, H, G3)
